# revision 1
# baseline (speedup 1.0000x reference)
"""RWKV WKV attention block on 8 Trainium2 NeuronCores.

Sharding: data-parallel over batch B=8 -> one batch element per core.

The time-mix tensors xk/xv/xr = tm*x_t + (1-tm)*x_{t-1} are pure input
preprocessing; they are computed on the host (numpy, bf16) and streamed in
directly, so the device spends its vector engines only on the WKV tail.

Per-core pipeline (T=2048 in chunks of Tc=512):
  K/V/R projections (PE bf16, fp32 PSUM accumulate) ->
  ek=exp(k), euk=exp(k+tf)=etf*ek (per-partition bias), e1=exp(-r)+1
  (all ACT; only Exp/Identity/Copy -> no act-table reloads) ->
  ekv=ek*v (DVE) -> WKV linear recurrence (fp32 DVE tensor_tensor_scan) ->
  numer = etf*ekv + num_prev   (DVE STT, in-place on ekv)
  denom = euk + den_prev       (Pool TT add, in-place on euk)
  dd    = e1p * denom          (Pool TT mult -> sigmoid gate folded:
                                out*sr = numer / (denom*(1+exp(-r))))
  rdd   = 1/dd                 (DVE reciprocal, in-place)
  wsr   = numer * rdd -> bf16  (Pool TT; every 4th jg on DVE for balance)
  -> output projection (PE bf16) -> ACT copy -> DMA out.
Engine busy per rep: PE ~221us (the bf16 1-col/cycle roofline and overall
bound; 4 DxD projections are irreducible), DVE ~102, ACT ~98, Pool ~98.
Weight/const DMA is hoisted before the body; chunk-0 activation DMA is queued
ahead of the 8.4MB weight DMA so the first matmuls start ~10us in.
All weights pre-transposed on host; no on-device transposes.
"""

import sys

for _p in ("/opt/trn_rl_repo", "/root/.axon_site/_ro/trn_rl_repo"):
    if _p not in sys.path:
        sys.path.append(_p)

import numpy as np

import concourse.bass as bass
import concourse.mybir as mybir
import concourse.tile as tile
from concourse.bass_utils import run_bass_kernel_spmd

F32 = mybir.dt.float32
BF16 = mybir.dt.bfloat16
MMDT = BF16
ALU = mybir.AluOpType
ACT_F = mybir.ActivationFunctionType

B, T, D = 8, 2048, 1024
P = 128
G = D // P          # 8 channel groups
TC = 512            # T chunk
NCH = T // TC       # 4 chunks
TS = TC // P        # 4 t-subtiles per chunk in the output projection


def _split_waits(nc, maxw=1):
    """walrus in this image rejects >1 sync-wait per instruction; move the
    excess onto preceding same-engine no-ops (semantically identical)."""
    for f in nc.m.functions:
        for bb in f.blocks:
            new_insts = []
            for ins in bb.instructions:
                si = ins.sync_info
                if si is not None and si.on_wait and len(si.on_wait) > maxw:
                    waits = list(si.on_wait)
                    extra, keep = waits[:-maxw], waits[-maxw:]
                    for i in range(0, len(extra), maxw):
                        nop = mybir.InstNoOp(name=f"{ins.name}-ws{i}", ins=[], outs=[])
                        nop.engine = ins.engine
                        nop.sync_info = mybir.SyncInfo(
                            on_wait=extra[i:i + maxw], on_update=[])
                        new_insts.append(nop)
                        nc.register_instruction(nop, overwrite=True)
                    si.on_wait = keep
                new_insts.append(ins)
            bb.instructions = new_insts


def _build_nc(reps=None):
    nc = bass.Bass()

    xkP = nc.declare_dram_parameter("xk", [P, G, T], BF16, isOutput=False)
    xvP = nc.declare_dram_parameter("xv", [P, G, T], BF16, isOutput=False)
    xrP = nc.declare_dram_parameter("xr", [P, G, T], BF16, isOutput=False)
    WkT = nc.declare_dram_parameter("WkT", [P, G, D], MMDT, isOutput=False)
    WvT = nc.declare_dram_parameter("WvT", [P, G, D], MMDT, isOutput=False)
    WrT = nc.declare_dram_parameter("WrT", [P, G, D], MMDT, isOutput=False)
    WoT = nc.declare_dram_parameter("WoT", [P, G, D], MMDT, isOutput=False)
    ew_p = nc.declare_dram_parameter("ew", [P, G], F32, isOutput=False)
    etf_p = nc.declare_dram_parameter("etf", [P, G], F32, isOutput=False)
    tf_p = nc.declare_dram_parameter("tf", [P, G], F32, isOutput=False)
    ln_p = nc.declare_dram_parameter("lnum", [P, G], F32, isOutput=False)
    ld_p = nc.declare_dram_parameter("lden", [P, G], F32, isOutput=False)
    out_p = nc.declare_dram_parameter("out", [T, D], F32, isOutput=True)

    with tile.TileContext(nc) as tc:
        with tc.tile_pool(name="wts", bufs=1) as wts, \
             tc.tile_pool(name="consts", bufs=1) as consts, \
             tc.tile_pool(name="mix", bufs=2) as mixp, \
             tc.tile_pool(name="scan", bufs=1) as scanp, \
             tc.tile_pool(name="tr", bufs=3) as tr, \
             tc.tile_pool(name="wsrp", bufs=2) as wsrp, \
             tc.tile_pool(name="wop", bufs=1) as wop, \
             tc.tile_pool(name="outp", bufs=3) as outp, \
             tc.tile_pool(name="pskvr", bufs=2, space="PSUM") as pskvr, \
             tc.tile_pool(name="psout", bufs=2, space="PSUM") as psout:

            def load_one(nm, par, c):
                t = mixp.tile([P, G, TC], BF16, tag=nm)
                nc.sync.dma_start(t[:], par[:, :, c * TC:(c + 1) * TC])
                return t

            def load_mix(c):
                return [load_one(nm, par, c)
                        for nm, par in (("xk", xkP), ("xv", xvP), ("xr", xrP))]

            def emit_weights(mix_first):
                ew_sb = consts.tile([P, G], F32, tag="ew")
                etf_sb = consts.tile([P, G], F32, tag="etf")
                tf_sb = consts.tile([P, G], F32, tag="tf")
                ln_sb = consts.tile([P, G], F32, tag="ln")
                ld_sb = consts.tile([P, G], F32, tag="ld")
                nc.sync.dma_start(ew_sb[:], ew_p[:])
                nc.sync.dma_start(etf_sb[:], etf_p[:])
                nc.sync.dma_start(tf_sb[:], tf_p[:])
                nc.sync.dma_start(ln_sb[:], ln_p[:])
                nc.sync.dma_start(ld_sb[:], ld_p[:])

                wk = wts.tile([P, G, D], MMDT, tag="wk")
                wv = wts.tile([P, G, D], MMDT, tag="wv")
                wr = wts.tile([P, G, D], MMDT, tag="wr")
                for ig in range(G):
                    nc.sync.dma_start(wk[:, ig], WkT[:, ig])
                mix_first.append(load_one("xv", xvP, 0))
                for ig in range(G):
                    nc.sync.dma_start(wv[:, ig], WvT[:, ig])
                mix_first.append(load_one("xr", xrP, 0))
                for ig in range(G):
                    nc.sync.dma_start(wr[:, ig], WrT[:, ig])

                # persistent scan state buffers: [p, jg, 1+TC]; col 0 = carry-in
                numb = scanp.tile([P, G, 1 + TC], F32, tag="numb")
                denb = scanp.tile([P, G, 1 + TC], F32, tag="denb")

                wo = wop.tile([P, G, D], MMDT, tag="wo")
                for ig in range(G):
                    nc.sync.dma_start(wo[:, ig], WoT[:, ig])
                return (ew_sb, etf_sb, tf_sb, ln_sb, ld_sb,
                        wk, wv, wr, wo, numb, denb)

            def emit_body(state, mix_first):
                (ew_sb, etf_sb, tf_sb, ln_sb, ld_sb,
                 wk, wv, wr, wo, numb, denb) = state
                mix_next = mix_first
                pend_o = None

                def emit_opass(wsr, t0):
                    # O-pass for the PREVIOUS chunk: its wsr is complete by
                    # emission time, so PE never stalls on the WKV tail.
                    for dt in range(2):
                        for ts in range(TS):
                            ops = psout.tile([P, 512], F32, tag="ops")
                            for jg in range(G):
                                nc.tensor.matmul(
                                    ops[:], wsr[:, jg, bass.ts(ts, P)],
                                    wo[:, jg, bass.ts(dt, 512)],
                                    start=(jg == 0), stop=(jg == G - 1))
                            ob = outp.tile([P, 512], F32, tag="ob")
                            nc.scalar.copy(ob[:], ops[:])
                            nc.sync.dma_start(
                                out_p[t0 + ts * P:t0 + (ts + 1) * P,
                                      bass.ts(dt, 512)], ob[:])

                for c in range(NCH):
                    t0 = c * TC

                    xk, xv, xr = mix_next
                    if c + 1 < NCH:
                        mix_next = load_mix(c + 1)

                    # carry-in columns for all jg at once (strided copy)
                    if c == 0:
                        nc.vector.tensor_copy(numb[:, :, 0], ln_sb[:, :])
                        nc.vector.tensor_copy(denb[:, :, 0], ld_sb[:, :])
                    else:
                        nc.vector.tensor_copy(numb[:, :, 0], numb[:, :, TC])
                        nc.vector.tensor_copy(denb[:, :, 0], denb[:, :, TC])

                    # ---- K/V/R projections + WKV tail -> wsr ----
                    wsr = wsrp.tile([P, G, TC], MMDT, tag="wsr")
                    for jg in range(G):
                        jsl = bass.ts(jg, P)
                        kps = pskvr.tile([P, TC], F32, tag="kps")
                        for ig in range(G):
                            nc.tensor.matmul(kps[:], wk[:, ig, jsl], xk[:, ig],
                                             start=(ig == 0), stop=(ig == G - 1))
                        vps = pskvr.tile([P, TC], F32, tag="vps")
                        for ig in range(G):
                            nc.tensor.matmul(vps[:], wv[:, ig, jsl], xv[:, ig],
                                             start=(ig == 0), stop=(ig == G - 1))
                        rps = pskvr.tile([P, TC], F32, tag="rps")
                        for ig in range(G):
                            nc.tensor.matmul(rps[:], wr[:, ig, jsl], xr[:, ig],
                                             start=(ig == 0), stop=(ig == G - 1))

                        ekt = tr.tile([P, TC], F32, tag="ek")
                        ek = ekt[:]
                        nc.scalar.activation(ek, kps[:], ACT_F.Exp)
                        # euk = exp(k + tf) = etf*ek  (ACT, per-partition bias)
                        eukt = tr.tile([P, TC], F32, tag="euk")
                        euk = eukt[:]
                        nc.scalar.activation(euk, kps[:], ACT_F.Exp,
                                             bias=tf_sb[:, jg:jg + 1])
                        e1t = tr.tile([P, TC], F32, tag="e1")
                        nc.scalar.activation(e1t[:], rps[:], ACT_F.Exp,
                                             scale=-1.0)
                        # e1 + 1 (ACT Identity-add, in-place)
                        nc.scalar.add(e1t[:], e1t[:], 1.0)
                        ekvt = tr.tile([P, TC], F32, tag="ekv")
                        nc.vector.tensor_mul(ekvt[:], ek, vps[:])

                        ewb = ew_sb[:, jg:jg + 1].to_broadcast([P, TC])
                        nc.vector.tensor_tensor_scan(
                            numb[:, jg, 1:1 + TC], ewb, ekvt[:],
                            numb[:, jg, 0:1], ALU.mult, ALU.add)
                        nc.vector.tensor_tensor_scan(
                            denb[:, jg, 1:1 + TC], ewb, ek,
                            denb[:, jg, 0:1], ALU.mult, ALU.add)

                        etfs = etf_sb[:, jg:jg + 1]
                        # numer = etf*ekv + num_prev  (in-place onto ekv, DVE)
                        nc.vector.scalar_tensor_tensor(
                            ekvt[:], ekvt[:], etfs, numb[:, jg, 0:TC],
                            ALU.mult, ALU.add)
                        # denom = euk + den_prev      (in-place onto euk, Pool)
                        nc.gpsimd.tensor_add(euk, euk, denb[:, jg, 0:TC])
                        # dd = (e1+1) * denom         (in-place onto euk, Pool)
                        nc.gpsimd.tensor_mul(euk, euk, e1t[:])
                        nc.vector.reciprocal(euk, euk)
                        # wsr = numer * (1/dd) -> bf16 (Pool; every 4th on DVE
                        # to balance engine load)
                        if jg % 4 == 3:
                            nc.vector.tensor_mul(wsr[:, jg], ekvt[:], euk)
                        else:
                            nc.gpsimd.tensor_mul(wsr[:, jg], ekvt[:], euk)

                    if pend_o is not None:
                        emit_opass(*pend_o)
                    pend_o = (wsr, t0)

                emit_opass(*pend_o)

            # startup order: each projection's chunk-0 activations are
            # queued immediately before its weights, so the first K chain
            # (xk0+Wk = 3.1MB) is ready ~5us earlier than a bulk load.
            mix_first = [load_one("xk", xkP, 0)]
            state = emit_weights(mix_first)
            if reps and reps > 1:
                # on-device repeat loop (timing only; kernel() uses reps=1)
                with tc.For_i(0, reps, 1):
                    emit_body(state, mix_first)
                    mix_first = load_mix(0)
            else:
                emit_body(state, mix_first)

    _split_waits(nc, 1)
    return nc


_NC_CACHE = None


def _get_nc():
    global _NC_CACHE
    if _NC_CACHE is None:
        _NC_CACHE = _build_nc()
    return _NC_CACHE


def _pg(v):
    """(D,) channel vector -> [P, G] with channel d = g*128 + p."""
    return np.ascontiguousarray(np.asarray(v, np.float32).reshape(G, P).T)


def _wt(w):
    """W (D_out, D_in) -> W.T tiled [P, G, D_out] (contraction on partitions)."""
    wt = np.asarray(w, np.float32).T  # (D_in, D_out)
    out = np.ascontiguousarray(wt.reshape(G, P, D).transpose(1, 0, 2))
    return out.astype(mybir.dt.np(MMDT))


def _mixT(xs, tm):
    """xs (T+1, D) fp32, tm (D,) -> bf16 [P, G, T] of tm*x_t + (1-tm)*x_{t-1}."""
    m = xs[1:] * tm + xs[:-1] * (1.0 - tm)          # (T, D)
    return np.ascontiguousarray(
        m.T.reshape(G, P, T).transpose(1, 0, 2)).astype(mybir.dt.np(BF16))


def kernel(x, last_x, last_num, last_den, time_decay, time_first,
           time_mix_k, time_mix_v, time_mix_r, Wk, Wv, Wr, Wo):
    x = np.asarray(x, np.float32)
    last_x = np.asarray(last_x, np.float32)
    last_num = np.asarray(last_num, np.float32)
    last_den = np.asarray(last_den, np.float32)

    ew = _pg(np.exp(-np.exp(np.asarray(time_decay, np.float64))))
    etf = _pg(np.exp(np.asarray(time_first, np.float64)))
    tf = _pg(time_first)
    tmk = np.asarray(time_mix_k, np.float32).reshape(-1)
    tmv = np.asarray(time_mix_v, np.float32).reshape(-1)
    tmr = np.asarray(time_mix_r, np.float32).reshape(-1)
    wkT, wvT, wrT, woT = _wt(Wk), _wt(Wv), _wt(Wr), _wt(Wo)

    in_maps = []
    for b in range(B):
        xs = np.concatenate([last_x[b], x[b]], axis=0)      # (T+1, D)
        in_maps.append({
            "xk": _mixT(xs, tmk), "xv": _mixT(xs, tmv), "xr": _mixT(xs, tmr),
            "WkT": wkT, "WvT": wvT, "WrT": wrT, "WoT": woT,
            "ew": ew, "etf": etf, "tf": tf,
            "lnum": _pg(last_num[b, 0]), "lden": _pg(last_den[b, 0]),
        })

    global _last_in_maps
    _last_in_maps = in_maps
    nc = _get_nc()
    res = run_bass_kernel_spmd(nc, in_maps, list(range(B)))
    return np.stack([res.results[b]["out"] for b in range(B)], axis=0)


_last_in_maps = None



# revision 8
# speedup vs baseline: 1.2221x; 1.2221x over previous
"""RWKV WKV attention block on 8 Trainium2 NeuronCores.

Sharding: data-parallel over batch B=8 -> one batch element per core.

The time-mix tensors xk/xv/xr = tm*x_t + (1-tm)*x_{t-1} are pure input
preprocessing; they are computed on the host (numpy, bf16) and streamed in
directly, so the device spends its vector engines only on the WKV tail.

Per-core pipeline (T=2048 in chunks of Tc=512):
  K/V/R projections (PE bf16, fp32 PSUM accumulate) ->
  ek=exp(k), euk=exp(k+tf)=etf*ek (per-partition bias), e1=exp(-r)+1
  (all ACT; only Exp/Identity/Copy -> no act-table reloads) ->
  ekv=ek*v (DVE) -> WKV linear recurrence (fp32 DVE tensor_tensor_scan) ->
  numer = etf*ekv + num_prev   (DVE STT, in-place on ekv)
  denom = euk + den_prev       (Pool TT add, in-place on euk)
  dd    = e1p * denom          (Pool TT mult -> sigmoid gate folded:
                                out*sr = numer / (denom*(1+exp(-r))))
  rdd   = 1/dd                 (DVE reciprocal, in-place)
  wsr   = numer * rdd -> bf16  (Pool TT; every 4th jg on DVE for balance)
  -> output projection (PE bf16) -> ACT copy -> DMA out.
Engine busy per rep: PE ~221us (the bf16 1-col/cycle roofline and overall
bound; 4 DxD projections are irreducible), DVE ~102, ACT ~98, Pool ~98.
Weight/const DMA is hoisted before the body; chunk-0 activation DMA is queued
ahead of the 8.4MB weight DMA so the first matmuls start ~10us in.
All weights pre-transposed on host; no on-device transposes.
"""

import sys

for _p in ("/opt/trn_rl_repo", "/root/.axon_site/_ro/trn_rl_repo"):
    if _p not in sys.path:
        sys.path.append(_p)

import numpy as np

import concourse.bass as bass
import concourse.mybir as mybir
import concourse.tile as tile
from concourse.bass_utils import run_bass_kernel_spmd

F32 = mybir.dt.float32
BF16 = mybir.dt.bfloat16
MMDT = BF16
ALU = mybir.AluOpType
ACT_F = mybir.ActivationFunctionType

B, T, D = 8, 2048, 1024
P = 128
G = D // P          # 8 channel groups
TC = 512            # T chunk
NCH = T // TC       # 4 chunks
TS = TC // P        # 4 t-subtiles per chunk in the output projection


def _split_waits(nc, maxw=1):
    """walrus in this image rejects >1 sync-wait per instruction; move the
    excess onto preceding same-engine no-ops (semantically identical)."""
    for f in nc.m.functions:
        for bb in f.blocks:
            new_insts = []
            for ins in bb.instructions:
                si = ins.sync_info
                if si is not None and si.on_wait and len(si.on_wait) > maxw:
                    waits = list(si.on_wait)
                    extra, keep = waits[:-maxw], waits[-maxw:]
                    for i in range(0, len(extra), maxw):
                        nop = mybir.InstNoOp(name=f"{ins.name}-ws{i}", ins=[], outs=[])
                        nop.engine = ins.engine
                        nop.sync_info = mybir.SyncInfo(
                            on_wait=extra[i:i + maxw], on_update=[])
                        new_insts.append(nop)
                        nc.register_instruction(nop, overwrite=True)
                    si.on_wait = keep
                new_insts.append(ins)
            bb.instructions = new_insts


def _build_nc(reps=None, unroll=False):
    nc = bass.Bass()

    xkP = nc.declare_dram_parameter("xk", [P, G, T], BF16, isOutput=False)
    xvP = nc.declare_dram_parameter("xv", [P, G, T], BF16, isOutput=False)
    xrP = nc.declare_dram_parameter("xr", [P, G, T], BF16, isOutput=False)
    WkT = nc.declare_dram_parameter("WkT", [P, G, D], MMDT, isOutput=False)
    WvT = nc.declare_dram_parameter("WvT", [P, G, D], MMDT, isOutput=False)
    WrT = nc.declare_dram_parameter("WrT", [P, G, D], MMDT, isOutput=False)
    WoT = nc.declare_dram_parameter("WoT", [P, G, D], MMDT, isOutput=False)
    ew_p = nc.declare_dram_parameter("ew", [P, G], F32, isOutput=False)
    etf_p = nc.declare_dram_parameter("etf", [P, G], F32, isOutput=False)
    tf_p = nc.declare_dram_parameter("tf", [P, G], F32, isOutput=False)
    ln_p = nc.declare_dram_parameter("lnum", [P, G], F32, isOutput=False)
    ld_p = nc.declare_dram_parameter("lden", [P, G], F32, isOutput=False)
    out_p = nc.declare_dram_parameter("out", [T, D], F32, isOutput=True)

    with tile.TileContext(nc) as tc:
        with tc.tile_pool(name="wts", bufs=1) as wts, \
             tc.tile_pool(name="consts", bufs=1) as consts, \
             tc.tile_pool(name="mix", bufs=2) as mixp, \
             tc.tile_pool(name="scan", bufs=1) as scanp, \
             tc.tile_pool(name="tr", bufs=3) as tr, \
             tc.tile_pool(name="wsrp", bufs=2) as wsrp, \
             tc.tile_pool(name="wop", bufs=1) as wop, \
             tc.tile_pool(name="outp", bufs=3) as outp, \
             tc.tile_pool(name="pskvr", bufs=2, space="PSUM") as pskvr, \
             tc.tile_pool(name="psout", bufs=2, space="PSUM") as psout:

            def load_one(nm, par, c):
                t = mixp.tile([P, G, TC], BF16, tag=nm)
                nc.sync.dma_start(t[:], par[:, :, c * TC:(c + 1) * TC])
                return t

            def load_mix(c):
                return [load_one(nm, par, c)
                        for nm, par in (("xk", xkP), ("xv", xvP), ("xr", xrP))]

            def emit_weights(mix_first):
                ew_sb = consts.tile([P, G], F32, tag="ew")
                etf_sb = consts.tile([P, G], F32, tag="etf")
                tf_sb = consts.tile([P, G], F32, tag="tf")
                ln_sb = consts.tile([P, G], F32, tag="ln")
                ld_sb = consts.tile([P, G], F32, tag="ld")
                nc.sync.dma_start(ew_sb[:], ew_p[:])
                nc.sync.dma_start(etf_sb[:], etf_p[:])
                nc.sync.dma_start(tf_sb[:], tf_p[:])
                nc.sync.dma_start(ln_sb[:], ln_p[:])
                nc.sync.dma_start(ld_sb[:], ld_p[:])

                wk = wts.tile([P, G, D], MMDT, tag="wk")
                wv = wts.tile([P, G, D], MMDT, tag="wv")
                wr = wts.tile([P, G, D], MMDT, tag="wr")
                for ig in range(G):
                    nc.sync.dma_start(wk[:, ig], WkT[:, ig])
                mix_first.append(load_one("xv", xvP, 0))
                for ig in range(G):
                    nc.sync.dma_start(wv[:, ig], WvT[:, ig])
                mix_first.append(load_one("xr", xrP, 0))
                for ig in range(G):
                    nc.sync.dma_start(wr[:, ig], WrT[:, ig])

                # persistent scan state buffers: [p, jg, 1+TC]; col 0 = carry-in
                numb = scanp.tile([P, G, 1 + TC], F32, tag="numb")
                denb = scanp.tile([P, G, 1 + TC], F32, tag="denb")

                wo = wop.tile([P, G, D], MMDT, tag="wo")
                for ig in range(G):
                    nc.sync.dma_start(wo[:, ig], WoT[:, ig])
                return (ew_sb, etf_sb, tf_sb, ln_sb, ld_sb,
                        wk, wv, wr, wo, numb, denb)

            def emit_body(state, mix_first):
                (ew_sb, etf_sb, tf_sb, ln_sb, ld_sb,
                 wk, wv, wr, wo, numb, denb) = state
                mix_next = mix_first
                pend_o = None
                # one-group software pipeline for the recip/wsr stage:
                # (ekvt=numer, ddt, wsr, jg) of the previous group
                prev = [None]

                def emit_stage2():
                    # 1/dd for the previous group via exp(-ln(dd)) on ACT --
                    # Ln and Exp share one act table set
                    # (natural_log_exp_and_others), and DVE's InstReciprocal
                    # (bit-exact divide, ~6 cycles/elem) leaves the DVE
                    # entirely. Emitted one group late so the Pool->ACT dd
                    # dependency always has a full group of slack (no FIFO
                    # head-of-line blocking).
                    if prev[0] is None:
                        return
                    p_ekvt, p_ddt, p_wsr, p_jg = prev[0]
                    lddt = tr.tile([P, TC], F32, tag="ldd")
                    nc.scalar.activation(lddt[:], p_ddt[:], ACT_F.Ln)
                    rddt = tr.tile([P, TC], F32, tag="rdd")
                    nc.scalar.activation(rddt[:], lddt[:], ACT_F.Exp,
                                         scale=-1.0)
                    nc.gpsimd.tensor_mul(p_wsr[:, p_jg], p_ekvt[:], rddt[:])
                    prev[0] = None

                def emit_opass(wsr, t0):
                    # O-pass for the PREVIOUS chunk: its wsr is complete by
                    # emission time, so PE never stalls on the WKV tail.
                    for dt in range(2):
                        for ts in range(TS):
                            ops = psout.tile([P, 512], F32, tag="ops")
                            for jg in range(G):
                                nc.tensor.matmul(
                                    ops[:], wsr[:, jg, bass.ts(ts, P)],
                                    wo[:, jg, bass.ts(dt, 512)],
                                    start=(jg == 0), stop=(jg == G - 1))
                            ob = outp.tile([P, 512], F32, tag="ob")
                            nc.scalar.copy(ob[:], ops[:])
                            nc.sync.dma_start(
                                out_p[t0 + ts * P:t0 + (ts + 1) * P,
                                      bass.ts(dt, 512)], ob[:])

                for c in range(NCH):
                    t0 = c * TC

                    xk, xv, xr = mix_next
                    if c + 1 < NCH:
                        mix_next = load_mix(c + 1)

                    # carry-in columns for all jg at once (strided copy)
                    if c == 0:
                        nc.vector.tensor_copy(numb[:, :, 0], ln_sb[:, :])
                        nc.vector.tensor_copy(denb[:, :, 0], ld_sb[:, :])
                    else:
                        nc.vector.tensor_copy(numb[:, :, 0], numb[:, :, TC])
                        nc.vector.tensor_copy(denb[:, :, 0], denb[:, :, TC])

                    # ---- K/V/R projections + WKV tail -> wsr ----
                    wsr = wsrp.tile([P, G, TC], MMDT, tag="wsr")
                    for jg in range(G):
                        jsl = bass.ts(jg, P)
                        kps = pskvr.tile([P, TC], F32, tag="kps")
                        for ig in range(G):
                            nc.tensor.matmul(kps[:], wk[:, ig, jsl], xk[:, ig],
                                             start=(ig == 0), stop=(ig == G - 1))
                        vps = pskvr.tile([P, TC], F32, tag="vps")
                        for ig in range(G):
                            nc.tensor.matmul(vps[:], wv[:, ig, jsl], xv[:, ig],
                                             start=(ig == 0), stop=(ig == G - 1))
                        rps = pskvr.tile([P, TC], F32, tag="rps")
                        for ig in range(G):
                            nc.tensor.matmul(rps[:], wr[:, ig, jsl], xr[:, ig],
                                             start=(ig == 0), stop=(ig == G - 1))

                        ekt = tr.tile([P, TC], F32, tag="ek")
                        ek = ekt[:]
                        nc.scalar.activation(ek, kps[:], ACT_F.Exp)
                        e1t = tr.tile([P, TC], F32, tag="e1")
                        nc.scalar.activation(e1t[:], rps[:], ACT_F.Exp,
                                             scale=-1.0)
                        # e1 + 1 (ACT Identity-add, in-place)
                        nc.scalar.add(e1t[:], e1t[:], 1.0)
                        ekvt = tr.tile([P, TC], F32, tag="ekv")
                        nc.vector.tensor_mul(ekvt[:], ek, vps[:])
                        # previous group's recip+wsr slot here: after ekv has
                        # drained vps, before this group's scans
                        emit_stage2()

                        ewb = ew_sb[:, jg:jg + 1].to_broadcast([P, TC])
                        nc.vector.tensor_tensor_scan(
                            numb[:, jg, 1:1 + TC], ewb, ekvt[:],
                            numb[:, jg, 0:1], ALU.mult, ALU.add)
                        nc.vector.tensor_tensor_scan(
                            denb[:, jg, 1:1 + TC], ewb, ek,
                            denb[:, jg, 0:1], ALU.mult, ALU.add)

                        etfs = etf_sb[:, jg:jg + 1]
                        # numer = etf*ekv + num_prev  (in-place onto ekv, DVE)
                        nc.vector.scalar_tensor_tensor(
                            ekvt[:], ekvt[:], etfs, numb[:, jg, 0:TC],
                            ALU.mult, ALU.add)
                        # denom = etf*ek + den_prev   (DVE STT; euk never
                        # materialized -> kps has a single drain read.
                        # Pool rejects TensorScalarPtr on this ISA.)
                        ddt = tr.tile([P, TC], F32, tag="dd")
                        nc.vector.scalar_tensor_tensor(
                            ddt[:], ek, etfs, denb[:, jg, 0:TC],
                            ALU.mult, ALU.add)
                        # dd = (e1+1) * denom         (in-place, Pool)
                        nc.gpsimd.tensor_mul(ddt[:], ddt[:], e1t[:])
                        prev[0] = (ekvt, ddt, wsr, jg)

                    if pend_o is not None:
                        emit_opass(*pend_o)
                    pend_o = (wsr, t0)

                emit_stage2()
                emit_opass(*pend_o)

            # startup order: each projection's chunk-0 activations are
            # queued immediately before its weights, so the first K chain
            # (xk0+Wk = 3.1MB) is ready ~5us earlier than a bulk load.
            mix_first = [load_one("xk", xkP, 0)]
            state = emit_weights(mix_first)
            if reps and reps > 1 and unroll:
                # python-unrolled reps (CoreSim can't run For_i in no_exec)
                for _ in range(reps - 1):
                    emit_body(state, mix_first)
                    mix_first = load_mix(0)
                emit_body(state, mix_first)
            elif reps and reps > 1:
                # on-device repeat loop (timing only; kernel() uses reps=1)
                with tc.For_i(0, reps, 1):
                    emit_body(state, mix_first)
                    mix_first = load_mix(0)
            else:
                emit_body(state, mix_first)

    _split_waits(nc, 1)
    return nc


_NC_CACHE = None


def _get_nc():
    global _NC_CACHE
    if _NC_CACHE is None:
        _NC_CACHE = _build_nc()
    return _NC_CACHE


def _pg(v):
    """(D,) channel vector -> [P, G] with channel d = g*128 + p."""
    return np.ascontiguousarray(np.asarray(v, np.float32).reshape(G, P).T)


def _wt(w):
    """W (D_out, D_in) -> W.T tiled [P, G, D_out] (contraction on partitions)."""
    wt = np.asarray(w, np.float32).T  # (D_in, D_out)
    out = np.ascontiguousarray(wt.reshape(G, P, D).transpose(1, 0, 2))
    return out.astype(mybir.dt.np(MMDT))


def _mixT(xs, tm):
    """xs (T+1, D) fp32, tm (D,) -> bf16 [P, G, T] of tm*x_t + (1-tm)*x_{t-1}."""
    m = xs[1:] * tm + xs[:-1] * (1.0 - tm)          # (T, D)
    return np.ascontiguousarray(
        m.T.reshape(G, P, T).transpose(1, 0, 2)).astype(mybir.dt.np(BF16))


def kernel(x, last_x, last_num, last_den, time_decay, time_first,
           time_mix_k, time_mix_v, time_mix_r, Wk, Wv, Wr, Wo):
    x = np.asarray(x, np.float32)
    last_x = np.asarray(last_x, np.float32)
    last_num = np.asarray(last_num, np.float32)
    last_den = np.asarray(last_den, np.float32)

    ew = _pg(np.exp(-np.exp(np.asarray(time_decay, np.float64))))
    etf = _pg(np.exp(np.asarray(time_first, np.float64)))
    tf = _pg(time_first)
    tmk = np.asarray(time_mix_k, np.float32).reshape(-1)
    tmv = np.asarray(time_mix_v, np.float32).reshape(-1)
    tmr = np.asarray(time_mix_r, np.float32).reshape(-1)
    wkT, wvT, wrT, woT = _wt(Wk), _wt(Wv), _wt(Wr), _wt(Wo)

    in_maps = []
    for b in range(B):
        xs = np.concatenate([last_x[b], x[b]], axis=0)      # (T+1, D)
        in_maps.append({
            "xk": _mixT(xs, tmk), "xv": _mixT(xs, tmv), "xr": _mixT(xs, tmr),
            "WkT": wkT, "WvT": wvT, "WrT": wrT, "WoT": woT,
            "ew": ew, "etf": etf, "tf": tf,
            "lnum": _pg(last_num[b, 0]), "lden": _pg(last_den[b, 0]),
        })

    global _last_in_maps
    _last_in_maps = in_maps
    nc = _get_nc()
    res = run_bass_kernel_spmd(nc, in_maps, list(range(B)))
    return np.stack([res.results[b]["out"] for b in range(B)], axis=0)


_last_in_maps = None



# revision 12
# speedup vs baseline: 1.4282x; 1.1687x over previous
"""RWKV WKV attention block on 8 Trainium2 NeuronCores.

Sharding: data-parallel over batch B=8 -> one batch element per core.

The time-mix tensors xk/xv/xr = tm*x_t + (1-tm)*x_{t-1} are pure input
preprocessing; they are computed on the host (numpy, bf16) and streamed in
directly, so the device spends its vector engines only on the WKV tail.

Per-core pipeline (T=2048 in chunks of Tc=512):
  K/V/R projections (PE bf16, fp32 PSUM accumulate) ->
  ek=exp(k), euk=exp(k+tf)=etf*ek (per-partition bias), e1=exp(-r)+1
  (all ACT; only Exp/Identity/Copy -> no act-table reloads) ->
  ekv=ek*v (DVE) -> WKV linear recurrence (fp32 DVE tensor_tensor_scan) ->
  numer = etf*ekv + num_prev   (DVE STT, in-place on ekv)
  denom = euk + den_prev       (Pool TT add, in-place on euk)
  dd    = e1p * denom          (Pool TT mult -> sigmoid gate folded:
                                out*sr = numer / (denom*(1+exp(-r))))
  rdd   = 1/dd                 (DVE reciprocal, in-place)
  wsr   = numer * rdd -> bf16  (Pool TT; every 4th jg on DVE for balance)
  -> output projection (PE bf16) -> ACT copy -> DMA out.
Engine busy per rep: PE ~221us (the bf16 1-col/cycle roofline and overall
bound; 4 DxD projections are irreducible), DVE ~102, ACT ~98, Pool ~98.
Weight/const DMA is hoisted before the body; chunk-0 activation DMA is queued
ahead of the 8.4MB weight DMA so the first matmuls start ~10us in.
All weights pre-transposed on host; no on-device transposes.
"""

import sys

for _p in ("/opt/trn_rl_repo", "/root/.axon_site/_ro/trn_rl_repo"):
    if _p not in sys.path:
        sys.path.append(_p)

import numpy as np

import concourse.bass as bass
import concourse.mybir as mybir
import concourse.tile as tile
from concourse.bass_utils import run_bass_kernel_spmd

F32 = mybir.dt.float32
BF16 = mybir.dt.bfloat16
FP8 = mybir.dt.float8e4
MMDT = BF16
WRS = 32.0            # Wr pre-scale before e4m3 quantization
DRMODE = mybir.MatmulPerfMode.DoubleRow
ALU = mybir.AluOpType
ACT_F = mybir.ActivationFunctionType

B, T, D = 8, 2048, 1024
P = 128
G = D // P          # 8 channel groups
TC = 512            # T chunk
NCH = T // TC       # 4 chunks
TS = TC // P        # 4 t-subtiles per chunk in the output projection


def _split_waits(nc, maxw=1):
    """walrus in this image rejects >1 sync-wait per instruction; move the
    excess onto preceding same-engine no-ops (semantically identical)."""
    for f in nc.m.functions:
        for bb in f.blocks:
            new_insts = []
            for ins in bb.instructions:
                si = ins.sync_info
                if si is not None and si.on_wait and len(si.on_wait) > maxw:
                    waits = list(si.on_wait)
                    extra, keep = waits[:-maxw], waits[-maxw:]
                    for i in range(0, len(extra), maxw):
                        nop = mybir.InstNoOp(name=f"{ins.name}-ws{i}", ins=[], outs=[])
                        nop.engine = ins.engine
                        nop.sync_info = mybir.SyncInfo(
                            on_wait=extra[i:i + maxw], on_update=[])
                        new_insts.append(nop)
                        nc.register_instruction(nop, overwrite=True)
                    si.on_wait = keep
                new_insts.append(ins)
            bb.instructions = new_insts


def _build_nc(reps=None, unroll=False):
    nc = bass.Bass()

    xkP = nc.declare_dram_parameter("xk", [P, G, T], BF16, isOutput=False)
    xvP = nc.declare_dram_parameter("xv", [P, G, T], BF16, isOutput=False)
    xrP = nc.declare_dram_parameter("xr", [P, G, T], FP8, isOutput=False)
    WkT = nc.declare_dram_parameter("WkT", [P, G, D], MMDT, isOutput=False)
    WvT = nc.declare_dram_parameter("WvT", [P, G, D], MMDT, isOutput=False)
    WrT = nc.declare_dram_parameter("WrT", [P, G, D], FP8, isOutput=False)
    WoT = nc.declare_dram_parameter("WoT", [P, G, D], MMDT, isOutput=False)
    ew_p = nc.declare_dram_parameter("ew", [P, G], F32, isOutput=False)
    etf_p = nc.declare_dram_parameter("etf", [P, G], F32, isOutput=False)
    tf_p = nc.declare_dram_parameter("tf", [P, G], F32, isOutput=False)
    ln_p = nc.declare_dram_parameter("lnum", [P, G], F32, isOutput=False)
    ld_p = nc.declare_dram_parameter("lden", [P, G], F32, isOutput=False)
    out_p = nc.declare_dram_parameter("out", [T, D], F32, isOutput=True)

    with tile.TileContext(nc) as tc:
        with tc.tile_pool(name="wts", bufs=1) as wts, \
             tc.tile_pool(name="consts", bufs=1) as consts, \
             tc.tile_pool(name="mix", bufs=2) as mixp, \
             tc.tile_pool(name="scan", bufs=1) as scanp, \
             tc.tile_pool(name="tr", bufs=3) as tr, \
             tc.tile_pool(name="wsrp", bufs=2) as wsrp, \
             tc.tile_pool(name="wop", bufs=1) as wop, \
             tc.tile_pool(name="outp", bufs=3) as outp, \
             tc.tile_pool(name="pskvr", bufs=2, space="PSUM") as pskvr, \
             tc.tile_pool(name="psout", bufs=2, space="PSUM") as psout:

            def load_one(nm, par, c):
                t = mixp.tile([P, G, TC], FP8 if nm == "xr" else BF16, tag=nm)
                nc.sync.dma_start(t[:], par[:, :, c * TC:(c + 1) * TC])
                return t

            def load_mix(c):
                return [load_one(nm, par, c)
                        for nm, par in (("xk", xkP), ("xv", xvP), ("xr", xrP))]

            def emit_weights(mix_first):
                ew_sb = consts.tile([P, G], F32, tag="ew")
                etf_sb = consts.tile([P, G], F32, tag="etf")
                tf_sb = consts.tile([P, G], F32, tag="tf")
                ln_sb = consts.tile([P, G], F32, tag="ln")
                ld_sb = consts.tile([P, G], F32, tag="ld")
                nc.sync.dma_start(ew_sb[:], ew_p[:])
                nc.sync.dma_start(etf_sb[:], etf_p[:])
                nc.sync.dma_start(tf_sb[:], tf_p[:])
                nc.sync.dma_start(ln_sb[:], ln_p[:])
                nc.sync.dma_start(ld_sb[:], ld_p[:])

                wk = wts.tile([P, G, D], MMDT, tag="wk")
                wv = wts.tile([P, G, D], MMDT, tag="wv")
                wr = wts.tile([P, G, D], FP8, tag="wr")
                for ig in range(G):
                    nc.sync.dma_start(wk[:, ig], WkT[:, ig])
                mix_first.append(load_one("xv", xvP, 0))
                for ig in range(G):
                    nc.sync.dma_start(wv[:, ig], WvT[:, ig])
                mix_first.append(load_one("xr", xrP, 0))
                for ig in range(G):
                    nc.sync.dma_start(wr[:, ig], WrT[:, ig])

                # persistent scan state buffers: [p, jg, 1+TC]; col 0 = carry-in
                numb = scanp.tile([P, G, 1 + TC], F32, tag="numb")
                denb = scanp.tile([P, G, 1 + TC], F32, tag="denb")

                wo = wop.tile([P, G, D], MMDT, tag="wo")
                for ig in range(G):
                    nc.sync.dma_start(wo[:, ig], WoT[:, ig])
                return (ew_sb, etf_sb, tf_sb, ln_sb, ld_sb,
                        wk, wv, wr, wo, numb, denb)

            def emit_body(state, mix_first):
                (ew_sb, etf_sb, tf_sb, ln_sb, ld_sb,
                 wk, wv, wr, wo, numb, denb) = state
                mix_next = mix_first
                pend_o = None
                # one-group software pipeline for the recip/wsr stage:
                # (ekvt=numer, ddt, wsr, jg) of the previous group
                prev = [None]

                def emit_stage2():
                    # 1/dd for the previous group via exp(-ln(dd)) on ACT --
                    # Ln and Exp share one act table set
                    # (natural_log_exp_and_others), and DVE's InstReciprocal
                    # (bit-exact divide, ~6 cycles/elem) leaves the DVE
                    # entirely. Emitted one group late so the Pool->ACT dd
                    # dependency always has a full group of slack (no FIFO
                    # head-of-line blocking).
                    if prev[0] is None:
                        return
                    p_ekvt, p_ddt, p_wsr, p_jg = prev[0]
                    lddt = tr.tile([P, TC], F32, tag="ldd")
                    nc.scalar.activation(lddt[:], p_ddt[:], ACT_F.Ln)
                    rddt = tr.tile([P, TC], F32, tag="rdd")
                    nc.scalar.activation(rddt[:], lddt[:], ACT_F.Exp,
                                         scale=-1.0)
                    nc.gpsimd.tensor_mul(p_wsr[:, p_jg], p_ekvt[:], rddt[:])
                    prev[0] = None

                def emit_opass(wsr, t0):
                    # O-pass for the PREVIOUS chunk: its wsr is complete by
                    # emission time, so PE never stalls on the WKV tail.
                    for dt in range(2):
                        for ts in range(TS):
                            ops = psout.tile([P, 512], F32, tag="ops")
                            for jg in range(G):
                                nc.tensor.matmul(
                                    ops[:], wsr[:, jg, bass.ts(ts, P)],
                                    wo[:, jg, bass.ts(dt, 512)],
                                    start=(jg == 0), stop=(jg == G - 1))
                            ob = outp.tile([P, 512], F32, tag="ob")
                            nc.scalar.copy(ob[:], ops[:])
                            nc.sync.dma_start(
                                out_p[t0 + ts * P:t0 + (ts + 1) * P,
                                      bass.ts(dt, 512)], ob[:])

                for c in range(NCH):
                    t0 = c * TC

                    xk, xv, xr = mix_next
                    if c + 1 < NCH:
                        mix_next = load_mix(c + 1)

                    # carry-in columns for all jg at once (strided copy)
                    if c == 0:
                        nc.vector.tensor_copy(numb[:, :, 0], ln_sb[:, :])
                        nc.vector.tensor_copy(denb[:, :, 0], ld_sb[:, :])
                    else:
                        nc.vector.tensor_copy(numb[:, :, 0], numb[:, :, TC])
                        nc.vector.tensor_copy(denb[:, :, 0], denb[:, :, TC])

                    # ---- K/V/R projections + WKV tail -> wsr ----
                    wsr = wsrp.tile([P, G, TC], MMDT, tag="wsr")
                    for jg in range(G):
                        jsl = bass.ts(jg, P)
                        kps = pskvr.tile([P, TC], F32, tag="kps")
                        for ig in range(G):
                            nc.tensor.matmul(kps[:], wk[:, ig, jsl], xk[:, ig],
                                             start=(ig == 0), stop=(ig == G - 1))
                        vps = pskvr.tile([P, TC], F32, tag="vps")
                        for ig in range(G):
                            nc.tensor.matmul(vps[:], wv[:, ig, jsl], xv[:, ig],
                                             start=(ig == 0), stop=(ig == G - 1))
                        rps = pskvr.tile([P, TC], F32, tag="rps")
                        for pr in range(G // 2):
                            nc.tensor.matmul(
                                rps[:], wr[:, 2 * pr:2 * pr + 2, jsl],
                                xr[:, 2 * pr:2 * pr + 2],
                                start=(pr == 0), stop=(pr == G // 2 - 1),
                                perf_mode=DRMODE)

                        ekt = tr.tile([P, TC], F32, tag="ek")
                        ek = ekt[:]
                        nc.scalar.activation(ek, kps[:], ACT_F.Exp)
                        e1t = tr.tile([P, TC], F32, tag="e1")
                        nc.scalar.activation(e1t[:], rps[:], ACT_F.Exp,
                                             scale=-1.0 / WRS)
                        # e1 + 1 (ACT Identity-add, in-place)
                        nc.scalar.add(e1t[:], e1t[:], 1.0)
                        ekvt = tr.tile([P, TC], F32, tag="ekv")
                        nc.vector.tensor_mul(ekvt[:], ek, vps[:])
                        # previous group's recip+wsr slot here: after ekv has
                        # drained vps, before this group's scans
                        emit_stage2()

                        ewb = ew_sb[:, jg:jg + 1].to_broadcast([P, TC])
                        nc.vector.tensor_tensor_scan(
                            numb[:, jg, 1:1 + TC], ewb, ekvt[:],
                            numb[:, jg, 0:1], ALU.mult, ALU.add)
                        nc.vector.tensor_tensor_scan(
                            denb[:, jg, 1:1 + TC], ewb, ek,
                            denb[:, jg, 0:1], ALU.mult, ALU.add)

                        etfs = etf_sb[:, jg:jg + 1]
                        # numer = etf*ekv + num_prev  (in-place onto ekv, DVE)
                        nc.vector.scalar_tensor_tensor(
                            ekvt[:], ekvt[:], etfs, numb[:, jg, 0:TC],
                            ALU.mult, ALU.add)
                        # denom = etf*ek + den_prev   (DVE STT; euk never
                        # materialized -> kps has a single drain read.
                        # Pool rejects TensorScalarPtr on this ISA.)
                        ddt = tr.tile([P, TC], F32, tag="dd")
                        nc.vector.scalar_tensor_tensor(
                            ddt[:], ek, etfs, denb[:, jg, 0:TC],
                            ALU.mult, ALU.add)
                        # dd = (e1+1) * denom         (in-place, Pool)
                        nc.gpsimd.tensor_mul(ddt[:], ddt[:], e1t[:])
                        prev[0] = (ekvt, ddt, wsr, jg)

                    if pend_o is not None:
                        emit_opass(*pend_o)
                    pend_o = (wsr, t0)

                emit_stage2()
                emit_opass(*pend_o)

            # startup order: each projection's chunk-0 activations are
            # queued immediately before its weights, so the first K chain
            # (xk0+Wk = 3.1MB) is ready ~5us earlier than a bulk load.
            mix_first = [load_one("xk", xkP, 0)]
            state = emit_weights(mix_first)
            if reps and reps > 1 and unroll:
                # python-unrolled reps (CoreSim can't run For_i in no_exec)
                for _ in range(reps - 1):
                    emit_body(state, mix_first)
                    mix_first = load_mix(0)
                emit_body(state, mix_first)
            elif reps and reps > 1:
                # on-device repeat loop (timing only; kernel() uses reps=1)
                with tc.For_i(0, reps, 1):
                    emit_body(state, mix_first)
                    mix_first = load_mix(0)
            else:
                emit_body(state, mix_first)

    _split_waits(nc, 1)
    return nc


_NC_CACHE = None


def _get_nc():
    global _NC_CACHE
    if _NC_CACHE is None:
        _NC_CACHE = _build_nc()
    return _NC_CACHE


def _pg(v):
    """(D,) channel vector -> [P, G] with channel d = g*128 + p."""
    return np.ascontiguousarray(np.asarray(v, np.float32).reshape(G, P).T)


def _wt(w):
    """W (D_out, D_in) -> W.T tiled [P, G, D_out] (contraction on partitions)."""
    wt = np.asarray(w, np.float32).T  # (D_in, D_out)
    out = np.ascontiguousarray(wt.reshape(G, P, D).transpose(1, 0, 2))
    return out.astype(mybir.dt.np(MMDT))


def _wt8(w):
    """Like _wt but pre-scaled by WRS and quantized to e4m3 (for DoubleRow)."""
    wt = np.asarray(w, np.float32).T * WRS
    out = np.ascontiguousarray(wt.reshape(G, P, D).transpose(1, 0, 2))
    return out.astype(mybir.dt.np(FP8))


def _mixT(xs, tm, dt=None):
    """xs (T+1, D) fp32, tm (D,) -> [P, G, T] of tm*x_t + (1-tm)*x_{t-1}."""
    m = xs[1:] * tm + xs[:-1] * (1.0 - tm)          # (T, D)
    return np.ascontiguousarray(
        m.T.reshape(G, P, T).transpose(1, 0, 2)).astype(
            mybir.dt.np(dt if dt is not None else BF16))


def kernel(x, last_x, last_num, last_den, time_decay, time_first,
           time_mix_k, time_mix_v, time_mix_r, Wk, Wv, Wr, Wo):
    x = np.asarray(x, np.float32)
    last_x = np.asarray(last_x, np.float32)
    last_num = np.asarray(last_num, np.float32)
    last_den = np.asarray(last_den, np.float32)

    ew = _pg(np.exp(-np.exp(np.asarray(time_decay, np.float64))))
    etf = _pg(np.exp(np.asarray(time_first, np.float64)))
    tf = _pg(time_first)
    tmk = np.asarray(time_mix_k, np.float32).reshape(-1)
    tmv = np.asarray(time_mix_v, np.float32).reshape(-1)
    tmr = np.asarray(time_mix_r, np.float32).reshape(-1)
    wkT, wvT, woT = _wt(Wk), _wt(Wv), _wt(Wo)
    wrT = _wt8(Wr)

    in_maps = []
    for b in range(B):
        xs = np.concatenate([last_x[b], x[b]], axis=0)      # (T+1, D)
        in_maps.append({
            "xk": _mixT(xs, tmk), "xv": _mixT(xs, tmv),
            "xr": _mixT(xs, tmr, FP8),
            "WkT": wkT, "WvT": wvT, "WrT": wrT, "WoT": woT,
            "ew": ew, "etf": etf, "tf": tf,
            "lnum": _pg(last_num[b, 0]), "lden": _pg(last_den[b, 0]),
        })

    global _last_in_maps
    _last_in_maps = in_maps
    nc = _get_nc()
    res = run_bass_kernel_spmd(nc, in_maps, list(range(B)))
    return np.stack([res.results[b]["out"] for b in range(B)], axis=0)


_last_in_maps = None

